# revision 3
# baseline (speedup 1.0000x reference)
"""Trainium2 Bass kernel for strict-causal (pixelSNAIL) attention, v2.

Problem: B=8, H=W=64 (N=4096), Ck=64, Cv=128, fp32.
    out[b] = softmax(mask(q@k^T/sqrt(Ck))) @ v   with strictly-causal mask
    (pixel i attends only to j < i; row 0 gets all-zero output).

Sharding: data-parallel over batch - one batch per NeuronCore, 8 cores.

Per-core algorithm (v2: transposed-score formulation):
  - Cast q,k to bf16, PE-transpose to qT,kT [64, N].
  - For each q-chunk of 512 rows, for each pair of k-tiles (ja, jb):
      S^T[k, q] = kT_j^T @ qT_chunk  (bf16 matmuls into PSUM, causal-trimmed,
      packed side by side; diagonal tile gets a -1e9 upper-incl-diag bias
      via a constant matmul in the same accumulation group)
      P^T = exp(0.125 * S^T)  on ScalarE, PSUM -> SBUF bf16 (one instr/pair)
      O[q, 0:128] += P^T_tile^T @ [V_j | 1]  (bf16, PSUM accumulate; the
      appended ones column makes col 128 the softmax row-sum - no separate
      row-sum pass and no P transposes anywhere)
  - Chunk tail: recip = 1/rowsum (DVE, from PSUM), o_sb = O * recip, DMA out.
"""

import os
import sys

sys.path.insert(0, "/opt/trn_rl_repo")

import numpy as np

import concourse.bass as bass
import concourse.bacc as bacc
import concourse.mybir as mybir
import concourse.tile as tile
from concourse.bass_utils import run_bass_kernel_spmd
from concourse.masks import make_identity

F32 = mybir.dt.float32
BF16 = mybir.dt.bfloat16

B, H, W, CK, CV = 8, 64, 64, 64, 128
N = H * W            # 4096
NT = N // 128        # 32 k-tiles / q-tiles
NCHUNK = N // 512    # 8 q-chunks
NEG = 1e9
SCALE = 1.0 / np.sqrt(CK)


def build_kernel(repeats=1):
    nc = bacc.Bacc("TRN2", target_bir_lowering=False, debug=False, num_devices=8)

    q = nc.dram_tensor("q", [N, CK], F32, kind="ExternalInput").ap()
    k = nc.dram_tensor("k", [N, CK], F32, kind="ExternalInput").ap()
    v = nc.dram_tensor("v", [N, CV], F32, kind="ExternalInput").ap()
    o = nc.dram_tensor("o", [N, CV], F32, kind="ExternalOutput").ap()

    q_r = q.rearrange("(t p) c -> p t c", p=128)
    k_r = k.rearrange("(t p) c -> p t c", p=128)
    v_r = v.rearrange("(t p) c -> p t c", p=128)

    with tile.TileContext(nc) as tc:
        with (
            tc.tile_pool(name="const", bufs=1) as const_pool,
            tc.tile_pool(name="stage", bufs=2) as stage_pool,
            tc.tile_pool(name="bf", bufs=2) as bf_pool,
            tc.tile_pool(name="pt", bufs=6) as pt_pool,
            tc.tile_pool(name="osb", bufs=2) as osb_pool,
            tc.tile_pool(name="stats", bufs=2) as stats_pool,
            tc.tile_pool(name="ps", bufs=2, space="PSUM") as ps,
        ):
            def emit_body():
                # ---- constants ----
                ident_bf = const_pool.tile([128, 128], BF16, tag="ident")
                make_identity(nc, ident_bf[:])
                # tri2[c, kk] = -NEG where kk >= c (upper incl diag): added to
                # S^T[k, q'] it masks k >= q' (strict causality) on the
                # diagonal 128x128 tile, via matmul(out, tri2, I) in the same
                # PSUM accumulation group as the score matmul.
                tri2_bf = const_pool.tile([128, 128], BF16, tag="tri2")
                nc.gpsimd.memset(tri2_bf[:], 0.0)
                nc.gpsimd.affine_select(
                    out=tri2_bf[:],
                    in_=tri2_bf[:],
                    compare_op=mybir.AluOpType.is_gt,  # keep 0 where c-kk > 0
                    fill=-NEG,
                    base=0,
                    pattern=[[-1, 128]],
                    channel_multiplier=1,
                )

                # warm the Exp activation table while DMAs run
                warm = stats_pool.tile([128, 1], F32, tag="warm")
                nc.scalar.activation(
                    warm[:], ident_bf[:, :1],
                    mybir.ActivationFunctionType.Exp,
                )

                # ---- staging + DMA (issued in need-order) ----
                q_stg = stage_pool.tile([128, NT, CK], F32, tag="q_stage")
                k_stg = stage_pool.tile([128, NT, CK], F32, tag="k_stage")
                vstg = stage_pool.tile([128, NT, CV], F32, tag="v_stage")
                q_bf = bf_pool.tile([128, NT, CK], BF16, tag="q_bf")
                k_bf = bf_pool.tile([128, NT, CK], BF16, tag="k_bf")
                # padded to CV+2 so each row starts 4-byte aligned (260*j)
                v_ext = bf_pool.tile([128, NT, CV + 2], BF16, tag="v_ext")
                qT = bf_pool.tile([64, N], BF16, tag="qT")
                kT = bf_pool.tile([64, N], BF16, tag="kT")

                def dma_qk(g):
                    nc.sync.dma_start(
                        q_stg[:, 4 * g : 4 * (g + 1), :],
                        q_r[:, 4 * g : 4 * (g + 1), :],
                    )
                    nc.sync.dma_start(
                        k_stg[:, 4 * g : 4 * (g + 1), :],
                        k_r[:, 4 * g : 4 * (g + 1), :],
                    )

                def dma_v(d):
                    nc.sync.dma_start(
                        vstg[:, 8 * d : 8 * (d + 1), :],
                        v_r[:, 8 * d : 8 * (d + 1), :],
                    )

                dma_qk(0)
                dma_v(0)
                dma_qk(1)
                for g in range(2, 8):
                    dma_qk(g)
                    if g % 2 == 0:
                        dma_v(g // 2)

                nc.gpsimd.memset(v_ext[:, :, CV : CV + 1], 1.0)

                def cast_qk(g):
                    nc.vector.tensor_copy(
                        q_bf[:, 4 * g : 4 * (g + 1), :],
                        q_stg[:, 4 * g : 4 * (g + 1), :],
                    )
                    nc.vector.tensor_copy(
                        k_bf[:, 4 * g : 4 * (g + 1), :],
                        k_stg[:, 4 * g : 4 * (g + 1), :],
                    )

                def cast_v(d):
                    nc.vector.tensor_copy(
                        v_ext[:, 8 * d : 8 * (d + 1), :CV],
                        vstg[:, 8 * d : 8 * (d + 1), :],
                    )

                def make_qkt(g, src_bf, dst):
                    ptr = ps.tile([64, 512], BF16, tag="s", name="ptr")
                    for u in range(4):
                        nc.tensor.transpose(
                            ptr[:, 128 * u : 128 * (u + 1)],
                            src_bf[:, 4 * g + u, :],
                            ident_bf[:],
                        )
                    nc.vector.tensor_copy(dst[:, 512 * g : 512 * (g + 1)], ptr[:])

                cast_qk(0)
                cast_v(0)
                make_qkt(0, q_bf, qT)
                make_qkt(0, k_bf, kT)

                # ---- main loop over q-chunks ----
                carry = [None]

                def emit_carry():
                    if carry[0] is not None:
                        carry[0]()
                        carry[0] = None

                for c in range(NCHUNK):
                    # one PSUM bank per q-tile: an accumulation group may not
                    # share a bank with another concurrently-open group
                    opsum = [
                        ps.tile([128, CV + 1], F32, tag=f"o{t}", bufs=1,
                                name=f"o{t}")
                        for t in range(4)
                    ]
                    npairs = 2 * c + 2
                    for gp in range(npairs):
                        ja, jb = 2 * gp, 2 * gp + 1
                        offs = [max(0, 128 * (j - 4 * c)) for j in (ja, jb)]
                        base_a = 0
                        base_b = 512 - offs[0]
                        bases = [base_a, base_b]
                        tot = base_b + 512 - offs[1]

                        s_ps = ps.tile([128, 1024], F32, tag="s", name="s_ps")
                        for j, off, bs in zip((ja, jb), offs, bases):
                            diag = off > 0 or j == 4 * c
                            # split at PSUM bank boundaries (512 f32 cols) -
                            # a single matmul output must stay in one bank
                            seg = bs
                            end = bs + 512 - off
                            while seg < end:
                                seg_end = min(end, (seg // 512 + 1) * 512)
                                tri_here = diag and seg == bs
                                nc.tensor.matmul(
                                    s_ps[:, seg:seg_end],
                                    kT[:, 128 * j : 128 * (j + 1)],
                                    qT[:, 512 * c + off + (seg - bs)
                                       : 512 * c + off + (seg_end - bs)],
                                    start=True,
                                    stop=not tri_here,
                                )
                                if tri_here:
                                    nc.tensor.matmul(
                                        s_ps[:, bs : bs + 128],
                                        tri2_bf[:],
                                        ident_bf[:],
                                        start=False,
                                        stop=True,
                                    )
                                seg = seg_end

                        pT = pt_pool.tile([128, 1024], BF16, tag="pt", name="pT")
                        nc.scalar.activation(
                            pT[:, :tot],
                            s_ps[:, :tot],
                            mybir.ActivationFunctionType.Exp,
                            scale=SCALE,
                        )

                        emit_carry()

                        def pv(c=c, ja=ja, jb=jb, offs=offs, bases=bases,
                               pT=pT, opsum=opsum):
                            for j, off, bs in zip((ja, jb), offs, bases):
                                t0 = off // 128
                                for t in range(t0, 4):
                                    nc.tensor.matmul(
                                        opsum[t][:],
                                        pT[:, bs + 128 * t - off
                                           : bs + 128 * (t + 1) - off],
                                        v_ext[:, j, : CV + 1],
                                        start=(j == 0),
                                        stop=(j == 4 * c + t),
                                    )

                        carry[0] = pv

                    # prefetch next chunk's qT/kT group (+v cast) on PE/DVE
                    if c + 1 < NCHUNK:
                        cast_qk(c + 1)
                        if c % 2 == 0 and c // 2 + 1 < 4:
                            cast_v(c // 2 + 1)
                        make_qkt(c + 1, q_bf, qT)
                        make_qkt(c + 1, k_bf, kT)

                    emit_carry()

                    # ---- chunk tail: normalize + store ----
                    o_sb = osb_pool.tile([128, 4, CV], F32, tag="osb",
                                         name="o_sb")
                    for t in range(4):
                        ot = opsum[t]
                        recip = stats_pool.tile([128, 1], F32, tag=f"rc{t}",
                                                name=f"recip{t}")
                        if c == 0 and t == 0:
                            ssum = stats_pool.tile([128, 1], F32, tag="ssum")
                            nc.vector.tensor_scalar_add(
                                ssum[:], ot[:, CV : CV + 1], 1e-30
                            )
                            nc.vector.reciprocal(recip[:], ssum[:])
                        else:
                            nc.vector.reciprocal(recip[:], ot[:, CV : CV + 1])
                        nc.vector.tensor_scalar_mul(
                            o_sb[:, t, :], ot[:, :CV], recip[:]
                        )
                    nc.sync.dma_start(
                        o[512 * c : 512 * (c + 1), :].rearrange(
                            "(t p) c -> p t c", p=128
                        ),
                        o_sb[:],
                    )

            if repeats > 1:
                with tc.For_i(0, repeats, 1):
                    emit_body()
            else:
                emit_body()

    nc.compile()
    return nc


_NC_CACHE = None


def kernel(**inputs: np.ndarray) -> np.ndarray:
    global _NC_CACHE
    if _NC_CACHE is None:
        _NC_CACHE = build_kernel()
    nc = _NC_CACHE

    query = np.ascontiguousarray(inputs["query"], dtype=np.float32)
    key = np.ascontiguousarray(inputs["key"], dtype=np.float32)
    value = np.ascontiguousarray(inputs["value"], dtype=np.float32)

    in_maps = [
        {
            "q": query[b].reshape(N, CK),
            "k": key[b].reshape(N, CK),
            "v": value[b].reshape(N, CV),
        }
        for b in range(B)
    ]
    res = run_bass_kernel_spmd(nc, in_maps, list(range(B)))
    out = np.stack([res.results[b]["o"] for b in range(B)], axis=0)
    return out.reshape(B, H, W, CV)


if __name__ == "__main__":
    rng = np.random.default_rng(0)
    qq = rng.standard_normal((B, H, W, CK), dtype=np.float32)
    kk = rng.standard_normal((B, H, W, CK), dtype=np.float32)
    vv = rng.standard_normal((B, H, W, CV), dtype=np.float32)
    out = kernel(query=qq, key=kk, value=vv)
    print("out", out.shape, out.dtype, np.abs(out).mean())
